# revision 1
# baseline (speedup 1.0000x reference)
"""Multi-head attention (B=4,S=2048,H=1024,NH=16,D=64) on 8 trn2 cores.

Sharding: core c = (g, b) with g = c // 4 (head-group of 8 heads = 512 dims,
tensor parallel) and b = c % 4 (batch, data parallel). Each core computes a
partial output (its head-group's contribution to the final projection),
transposed: ot = (attn_out_g @ wo_g)^T of shape [H, S]. Host sums the two
group partials per batch and adds bias.

Math notes (host/device split):
  - k-proj bias bk drops out of softmax (adds a per-query constant along the
    key axis), so it is not applied on device.
  - v-proj bias bv commutes through normalized attention (rows of the score
    matrix sum to 1): its contribution is bv @ wo, folded into the output
    bias on the host.

On-device layout: everything is computed transposed (feature dim on
partitions, sequence on the free axis) so the softmax key-axis lands on
partitions. Scores S^T are built per head as K_h^T(stationary) x Q_h^T,
exp() runs on the scalar engine straight out of PSUM, and the ones-column
appended to V in the AV matmul yields the softmax denominators for free.

Schedule: the scalar-engine exp (256 x ~1.1us + ~0.2us issue overhead) is
the attention-phase throughput floor, so heads run sequentially (halves
live PSUM: scores 2x2 banks double-buffered + one 2-bank AV accumulator +
2 projection banks = 8) and the projection / output-projection matmuls are
drip-fed into the tensor engine's idle slots inside the attention kt-loop
("filler units"), keeping the PE continuously busy and in its fast
p-state. A head's trailing AV matmuls and PSUM drain are deferred until
after the NEXT head's first scores matmuls (cross-head/cross-chunk carry)
so the scalar engine never waits at boundaries. Softmax normalization is
batched per 1024-query chunk (denominator rows -> one approx-reciprocal ->
DRAM round-trip partition-broadcast -> multiplies split across DVE and
gpsimd), with rows 0-5 emitted inside the last head's kt-loop and the
final out-projection chunk bridging the PE over the rows-6-7 chain.
Contraction-tile DMAs load strip PAIRS via 3-d access patterns: the sync
engine's ~600ns/descriptor issue rate, not bandwidth, bounded the lead-in.
Output partials store as f16 (halves output traffic; host upcasts).
"""

import sys

if "/opt/trn_rl_repo" not in sys.path:
    sys.path.insert(0, "/opt/trn_rl_repo")

import numpy as np

B, S, H, NH, D = 4, 2048, 1024, 16, 64
G = 2  # head-group split across cores (tensor parallel axis)
GH = H // G  # 512 dims (8 heads) per group
NCORES = 8
SCALE = 1.0 / float(D) ** 0.5  # 1/8

KT = H // 128  # 8 contraction tiles for projections
MT = GH // 128  # 4 m-tiles = head pairs per group
NQC = S // 512  # 4 sequence chunks of 512
SQ = S // 128  # 16 key-sequence tiles
VW = D + 1  # 65: V columns + ones column per head

_CACHE = {}

# filler pump schedule: units (~2 matmuls each) per attention kt-iteration,
# by global iteration index within a qcp window (128 iters)
CFG = {
    "pt_bufs": 5,
    "av_delay": 2,
    "fill_fast_until": 40,  # 1 unit/kt before this iter (qcp0)
    "fill_slow_num": 7,  # then fill_slow_num units per fill_slow_den iters
    "fill_slow_den": 10,
}


def _build():
    import concourse.tile as tile
    from concourse import bacc, mybir

    F32 = mybir.dt.float32
    F16 = mybir.dt.float16
    AF = mybir.ActivationFunctionType
    OP = mybir.AluOpType

    nc = bacc.Bacc("TRN2", target_bir_lowering=False, debug=False)

    xq = nc.dram_tensor("xq", [H, S], F16, kind="ExternalInput")
    xk = nc.dram_tensor("xk", [H, S], F16, kind="ExternalInput")
    xv = nc.dram_tensor("xv", [H, S], F16, kind="ExternalInput")
    wqd = nc.dram_tensor("wq", [H, GH], F16, kind="ExternalInput")
    wkd = nc.dram_tensor("wk", [H, GH], F16, kind="ExternalInput")
    wvd = nc.dram_tensor("wv", [H, GH], F16, kind="ExternalInput")
    wod = nc.dram_tensor("wo", [GH, H], F32, kind="ExternalInput")
    bqd = nc.dram_tensor("bq", [GH], F32, kind="ExternalInput")
    otd = nc.dram_tensor("ot", [H, S], F16, kind="ExternalOutput")

    with tile.TileContext(nc) as tc:
        with (
            tc.tile_pool(name="res", bufs=1) as res,
            tc.tile_pool(name="rot", bufs=2) as rot,
            tc.tile_pool(name="pmm", bufs=2, space="PSUM") as pmm,
            tc.tile_pool(name="ppj", bufs=2, space="PSUM") as ppj,
            tc.tile_pool(name="pso", bufs=1, space="PSUM") as pso,
            tc.tile_pool(name="dsc", bufs=2, space="DRAM") as dsc,
        ):
            # ---- residents ----
            qhT = [
                res.tile([128, S], F16, tag=f"qhT{m}", name=f"qhT{m}")
                for m in range(MT)
            ]
            khT = [
                res.tile([128, S], F16, tag=f"khT{m}", name=f"khT{m}")
                for m in range(MT)
            ]
            # normalized attention output aliases onto qhT (a query chunk of
            # qhT[t] is dead once attention for that chunk finishes)
            oT = qhT
            # unnormalized attention out staging, one 1024-query chunk deep
            # (reused across qcp via WAR deps)
            oTu = [
                res.tile([128, 1024], F16, tag=f"oTu{t}", name=f"oTu{t}")
                for t in range(MT)
            ]
            dn = res.tile([8, 1024], F16, tag="dn", name="dn")
            bc = res.tile([128, 8 * 1024], F16, tag="bc", name="bc")
            vaug = res.tile([128, SQ * 8 * VW], F16, tag="vaug", name="vaug")
            wo_bf = [
                res.tile([128, H], F16, tag=f"wob{t}", name=f"wob{t}")
                for t in range(MT)
            ]
            bq_sb = res.tile([128, MT], F32, tag="bqsb", name="bq_sb")

            # ones columns of vaug (V slots are overwritten by the V proj);
            # memset on the (idle) gpsimd engine so the vector queue is free
            # for the first V-projection drains
            nc.gpsimd.memset(vaug, 1.0)

            # Contraction tiles load as PAIRS via one 3-d access-pattern DMA
            # (the sync engine's ~600ns/descriptor issue rate dominated the
            # lead-in with per-strip DMAs). xs entries are (tile, col_base):
            # strip kt lives in pair tile kt//2 at column (kt%2)*512.
            def load_w(wd):
                xs = []
                for kp in range(KT // 2):
                    wt = rot.tile([128, 2 * GH], F16, tag="w", bufs=12, name=f"w{kp}")
                    nc.sync.dma_start(
                        out=wt,
                        in_=wd.ap()[kp * 256 : (kp + 1) * 256, :].rearrange("(a p) c -> p a c", a=2),
                    )
                    xs.append((wt, 0))
                    xs.append((wt, GH))
                return xs

            def load_strips(xd, qc, tag, bufs):
                xs = []
                for kp in range(KT // 2):
                    st = rot.tile(
                        [128, 1024], F16, tag=tag, bufs=bufs, name=f"{tag}{kp}"
                    )
                    nc.sync.dma_start(
                        out=st,
                        in_=xd.ap()[
                            kp * 256 : (kp + 1) * 256, qc * 512 : (qc + 1) * 512
                        ].rearrange("(a p) c -> p a c", a=2),
                    )
                    xs.append((st, 0))
                    xs.append((st, 512))
                return xs

            # ---- V projection: vaug[kseq, head*65] (untransposed, f16).
            # wv / xv0 pair-loads interleave so the first matmul (needs wv
            # pair 0 + xv pair 0) waits for just two descriptors.
            def load_w_interleaved(wd, xd, qc, tag, bufs):
                ws, xs = [], []
                for kp in range(KT // 2):
                    wt = rot.tile([128, 2 * GH], F16, tag="w", bufs=12, name=f"w{kp}")
                    nc.sync.dma_start(
                        out=wt,
                        in_=wd.ap()[kp * 256 : (kp + 1) * 256, :].rearrange(
                            "(a p) c -> p a c", a=2
                        ),
                    )
                    ws.append((wt, 0))
                    ws.append((wt, GH))
                    st = rot.tile(
                        [128, 1024], F16, tag=tag, bufs=bufs, name=f"{tag}{kp}"
                    )
                    nc.sync.dma_start(
                        out=st,
                        in_=xd.ap()[
                            kp * 256 : (kp + 1) * 256, qc * 512 : (qc + 1) * 512
                        ].rearrange("(a p) c -> p a c", a=2),
                    )
                    xs.append((st, 0))
                    xs.append((st, 512))
                return ws, xs

            wv_sb, xv0 = load_w_interleaved(wvd, xv, 0, tag="xv", bufs=8)

            def v_proj(qc, xs):
                for sql in range(4):
                    sq = qc * 4 + sql
                    ps = ppj.tile([128, 512], F32, tag="pj", name=f"psv{sq}")
                    for kt in range(KT):
                        xt, xo = xs[kt]
                        wt, wo = wv_sb[kt]
                        nc.tensor.matmul(
                            ps,
                            lhsT=xt[:, xo + sql * 128 : xo + (sql + 1) * 128],
                            rhs=wt[:, wo : wo + GH],
                            start=(kt == 0),
                            stop=(kt == KT - 1),
                        )
                    # one strided copy drains all 8 heads' V slots (the
                    # 65-stride layout skips the ones columns)
                    base = sq * 8 * VW
                    nc.vector.tensor_copy(
                        vaug[:, base : base + 8 * VW].rearrange(
                            "p (h w) -> p h w", w=VW
                        )[:, :, 0:D],
                        ps.rearrange("p (h w) -> p h w", w=D),
                    )

            # ---- K / Q projection helpers (emitted piecewise by fillers) ----
            def kq_mtile(xs, w_sb, m, drain):
                """Generator: one K/Q projection m-tile as 4 yield-units of
                2 matmuls; drain(ps) emitted with the last unit."""
                ps = ppj.tile([128, 512], F32, tag="pj", name=f"pkq{m}")
                for kt in range(KT):
                    xt, xo = xs[kt]
                    wt, wo = w_sb[kt]
                    nc.tensor.matmul(
                        ps,
                        lhsT=wt[:, wo + m * 128 : wo + (m + 1) * 128],
                        rhs=xt[:, xo : xo + 512],
                        start=(kt == 0),
                        stop=(kt == KT - 1),
                    )
                    if kt % 2 == 1 and kt < KT - 1:
                        yield
                drain(ps)
                yield

            def k_drain(m, qc):
                def d(ps):
                    nc.vector.tensor_copy(khT[m][:, qc * 512 : (qc + 1) * 512], ps)

                return d

            def q_drain(m, qc):
                def d(ps):
                    nc.vector.tensor_scalar(
                        qhT[m][:, qc * 512 : (qc + 1) * 512],
                        ps,
                        bq_sb[:, m : m + 1],
                        None,
                        OP.add,
                    )

                return d

            # ---- output projection for one 512-wide sequence chunk.
            # tail=True allocates PSUM from the (by then idle) scores pool
            # with 4-deep rotation so osb drains never stall the PE ----
            def out_proj_mtile(qcc, m, tail=False):
                ps = ppj.tile([128, 512], F32, tag="pj", name=f"pso{m}")
                for t in range(MT):
                    nc.tensor.matmul(
                        ps,
                        lhsT=wo_bf[t][:, m * 128 : (m + 1) * 128],
                        rhs=oT[t][:, qcc * 512 : (qcc + 1) * 512],
                        start=(t == 0),
                        stop=(t == MT - 1),
                    )
                    if t == 1:
                        yield
                osb = rot.tile([128, 512], F16, tag="osb", bufs=3, name="osb")
                if tail:
                    # the scalar engine is idle after the last exp; draining
                    # there keeps DVE free for the final normalize chain so
                    # the proj-PSUM rotation never starves the PE
                    nc.scalar.activation(osb, ps, AF.Copy)
                else:
                    nc.vector.tensor_copy(osb, ps)
                nc.sync.dma_start(
                    out=otd.ap()[m * 128 : (m + 1) * 128, qcc * 512 : (qcc + 1) * 512],
                    in_=osb,
                )
                yield

            # ---- attention for ONE head (t = pair index, hh in {0,1}) over a
            # 1024-wide query chunk, with filler units pumped each kt.
            # Returns a flush closure (tail AV matmuls + PSUM drain) that the
            # NEXT head invokes right after its first scores matmuls, so the
            # scalar engine sees no gap at head boundaries. ----
            def attention_head(t, hh, qcp, pump, prev_flush, hooks={}):
                q0 = qcp * 1024
                hp = 64 * hh
                h_abs = 2 * t + hh
                ps_o = pso.tile([VW, 1024], F32, tag="o", name=f"pso{t}_{hh}")
                pending = []

                def emit_av(kt, pt_t):
                    vbase = kt * 8 * VW + h_abs * VW
                    for qch in range(2):
                        nc.tensor.matmul(
                            ps_o[:, qch * 512 : (qch + 1) * 512],
                            lhsT=vaug[:, vbase : vbase + VW],
                            rhs=pt_t[:, qch * 512 : (qch + 1) * 512],
                            start=(kt == 0),
                            stop=(kt == SQ - 1),
                        )

                for kt in range(SQ):
                    ps_s = pmm.tile([128, 1024], F32, tag="mm", name="pss")
                    for qch in range(2):
                        nc.tensor.matmul(
                            ps_s[:, qch * 512 : (qch + 1) * 512],
                            lhsT=khT[t][hp : hp + 64, kt * 128 : (kt + 1) * 128],
                            rhs=qhT[t][
                                hp : hp + 64, q0 + qch * 512 : q0 + (qch + 1) * 512
                            ],
                            start=True,
                            stop=True,
                        )
                    if kt == 0 and prev_flush is not None:
                        prev_flush()
                    if kt in hooks:
                        hooks[kt]()
                    pump()
                    pt_t = rot.tile(
                        [128, 1024], F16, tag="pt", bufs=CFG["pt_bufs"], name="pt"
                    )
                    nc.scalar.activation(pt_t, ps_s, AF.Exp, scale=SCALE)
                    pending.append((kt, pt_t))
                    if len(pending) > CFG["av_delay"]:
                        emit_av(*pending.pop(0))

                def flush():
                    for p in pending:
                        emit_av(*p)
                    # drain: V-rows -> oTu (via DMA, h1 shifts partitions
                    # 0-63 -> 64-127), denominator row -> dn[2t+hh]
                    stg = rot.tile([VW, 1024], F16, tag="stg", bufs=3, name="stg")
                    nc.vector.tensor_copy(stg, ps_o)
                    nc.sync.dma_start(out=oTu[t][hp : hp + 64, :], in_=stg[0:D, :])
                    nc.sync.dma_start(
                        out=dn[h_abs : h_abs + 1, :], in_=stg[D : D + 1, :]
                    )

                return flush

            # ---- batched softmax normalization for head-rows [r0, r1) of a
            # 1024-query chunk. Rows 0-5 are emitted mid-way through the last
            # head's kt-loop (their denominators are long since drained), so
            # the qcp tail only carries rows 6-7. The multiplies alternate
            # between the vector and gpsimd engines. ----
            dnf = res.tile([8, 1024], F32, tag="dnf", name="dnf")
            rcpf = res.tile([8, 1024], F32, tag="rcpf", name="rcpf")
            rcph = res.tile([8, 1024], F16, tag="rcph", name="rcph")

            def normalize_rows(qcp, r0, r1):
                # DVE ops need an aligned start partition: run the chain on
                # rows [0, r1) (free-size-bound, so the extra rows are free)
                # and slice only the DMAs/multiplies to [r0, r1).
                q0 = qcp * 1024
                nc.vector.tensor_copy(dnf[0:r1, :], dn[0:r1, :])
                nc.vector.reciprocal_approx_fast(rcpf[0:r1, :], dnf[0:r1, :])
                nc.vector.tensor_copy(rcph[0:r1, :], rcpf[0:r1, :])
                sc = dsc.tile([r1 - r0, 1024], F16, tag="sc", name="sc")
                nc.sync.dma_start(out=sc, in_=rcph[r0:r1, :])
                for r in range(r0, r1):
                    nc.sync.dma_start(
                        out=bc[:, r * 1024 : (r + 1) * 1024],
                        in_=sc[r - r0, :].partition_broadcast(128),
                    )
                for r in range(r0, r1):
                    t, hh = divmod(r, 2)
                    hp = 64 * hh
                    # rows 6-7 are on the qcp tail critical path: DVE is
                    # ~3x faster than gpsimd per multiply
                    eng = nc.vector if (r % 2 == 0 or r >= 6) else nc.gpsimd
                    eng.tensor_tensor(
                        oT[t][hp : hp + 64, q0 : q0 + 1024],
                        oTu[t][hp : hp + 64, :],
                        bc[hp : hp + 64, r * 1024 : (r + 1) * 1024],
                        OP.mult,
                    )

            # ================= emission =================
            # lead-in: all of V proj; K qc0 (all m); Q m0 for qc0,qc1.
            # DMA emission order matters: the sync engine issues descriptors
            # in order, so each projection's inputs are queued just before
            # first use and bulk prefetches ride behind them.
            v_proj(0, xv0)
            nc.sync.dma_start(
                out=bq_sb, in_=bqd.ap().rearrange("(m p) -> p m", p=128)
            )
            wk_sb = load_w(wkd)
            xsv = load_strips(xv, 1, tag="xv", bufs=8)
            v_proj(1, xsv)
            wq_sb = load_w(wqd)
            xsv = load_strips(xv, 2, tag="xv", bufs=8)
            v_proj(2, xsv)
            kstrips = {0: load_strips(xk, 0, tag="xk", bufs=12)}
            xsv = load_strips(xv, 3, tag="xv", bufs=8)
            v_proj(3, xsv)
            for qc in range(1, 4):
                kstrips[qc] = load_strips(xk, qc, tag="xk", bufs=12)
            qstrips = {qc: load_strips(xq, qc, tag="xq", bufs=9) for qc in range(2)}
            for m in range(MT):
                for _ in kq_mtile(kstrips[0], wk_sb, m, k_drain(m, 0)):
                    pass
            for qc in range(2):
                for _ in kq_mtile(qstrips[qc], wq_sb, 0, q_drain(0, qc)):
                    pass
            # wo staging rides behind the lead-in loads (needed from the
            # first out-proj filler, mid-qcp1)
            for t in range(MT):
                wos = rot.tile([128, H], F32, tag="wos", bufs=1, name=f"wos{t}")
                nc.sync.dma_start(out=wos, in_=wod.ap()[t * 128 : (t + 1) * 128, :])
                nc.vector.tensor_copy(wo_bf[t], wos)

            # filler unit stream for the qcp0 attention window, then qcp1
            def filler_stream_qcp0():
                # K m0 for qc1-3 (t0's kt consumption), then Q m / K m by
                # ascending deadline, then qcp1's first Q m-tiles
                for qc in range(1, 4):
                    yield from kq_mtile(kstrips[qc], wk_sb, 0, k_drain(0, qc))
                for m in range(1, MT):
                    for qc in range(2):
                        yield from kq_mtile(qstrips[qc], wq_sb, m, q_drain(m, qc))
                    for qc in range(1, 4):
                        yield from kq_mtile(kstrips[qc], wk_sb, m, k_drain(m, qc))
                # Q projection m0 for qcp1 (loads its strips here)
                for qc in range(2, 4):
                    qstrips[qc] = load_strips(xq, qc, tag="xq", bufs=9)
                for qc in range(2, 4):
                    yield from kq_mtile(qstrips[qc], wq_sb, 0, q_drain(0, qc))

            def filler_stream_qcp1():
                # qcc0's out-proj rides as fillers; qcc1 is held back as the
                # PE "bridge" over the final normalize chain
                for m in range(1, MT):
                    for qc in range(2, 4):
                        yield from kq_mtile(qstrips[qc], wq_sb, m, q_drain(m, qc))
                for m in range(H // 128):
                    yield from out_proj_mtile(0, m)

            def run_qcp(qcp, gen, flush):
                it = [0]

                def pump():
                    if qcp == 0:
                        if it[0] < CFG["fill_fast_until"]:
                            n = 1
                        else:
                            n = (
                                1
                                if (it[0] % CFG["fill_slow_den"])
                                < CFG["fill_slow_num"]
                                else 0
                            )
                    else:
                        n = 1 if it[0] % 3 == 0 else 0
                    for _ in range(n):
                        try:
                            next(gen)
                        except StopIteration:
                            break
                    it[0] += 1

                for i, (t, hh) in enumerate(
                    (t, hh) for t in range(MT) for hh in range(2)
                ):
                    hooks = {}
                    if i == 0 and qcp == 1:
                        # previous qcp's last two denominator rows (its final
                        # head flushed at this head's kt=0)
                        hooks[4] = lambda: normalize_rows(0, 6, 8)
                    if i == 7:
                        hooks[2] = lambda: normalize_rows(qcp, 0, 6)
                    flush = attention_head(t, hh, qcp, pump, flush, hooks)
                # drain remaining fillers
                for _ in gen:
                    pass
                return flush

            flush = run_qcp(0, filler_stream_qcp0(), None)
            flush = run_qcp(1, filler_stream_qcp1(), flush)
            flush()
            # tail: the final normalize chain runs on DVE/sync while the PE
            # bridges over it with qcc1's out-proj (qcp0 data, all ready)
            normalize_rows(1, 6, 8)
            for m in range(H // 128):
                for _ in out_proj_mtile(1, m, tail=True):
                    pass
            for qcc in range(2, 4):
                for m in range(H // 128):
                    for _ in out_proj_mtile(qcc, m, tail=True):
                        pass

    nc.compile()
    return nc


def _get_nc():
    if "nc" not in _CACHE:
        _CACHE["nc"] = _build()
    return _CACHE["nc"]


def make_in_maps(q, k, v, wq, wk, wv, wo, bq):
    q = np.asarray(q, np.float32)
    k = np.asarray(k, np.float32)
    v = np.asarray(v, np.float32)
    in_maps = []
    for c in range(NCORES):
        g, b = divmod(c, B)
        sl = slice(g * GH, (g + 1) * GH)
        in_maps.append(
            {
                "xq": np.ascontiguousarray(q[b].T).astype(np.float16),
                "xk": np.ascontiguousarray(k[b].T).astype(np.float16),
                "xv": np.ascontiguousarray(v[b].T).astype(np.float16),
                "wq": np.ascontiguousarray(np.asarray(wq, np.float32)[:, sl]).astype(np.float16),
                "wk": np.ascontiguousarray(np.asarray(wk, np.float32)[:, sl]).astype(np.float16),
                "wv": np.ascontiguousarray(np.asarray(wv, np.float32)[:, sl]).astype(np.float16),
                "wo": np.ascontiguousarray(np.asarray(wo, np.float32)[sl, :]),
                "bq": np.ascontiguousarray(np.asarray(bq, np.float32)[sl]),
            }
        )
    return in_maps


def assemble(per_core_ot, bv, wo, bo):
    bo_eff = (
        np.asarray(bo, np.float32)
        + np.asarray(bv, np.float32) @ np.asarray(wo, np.float32)
    )
    out = np.empty((B, S, H), np.float32)
    for b in range(B):
        out[b] = (
            per_core_ot[b].T.astype(np.float32)
            + per_core_ot[B + b].T.astype(np.float32)
            + bo_eff
        )
    return out


def kernel(q, k, v, wq, bq, wk, bk, wv, bv, wo, bo, _trace=False):
    from concourse.bass_utils import run_bass_kernel_spmd

    nc = _get_nc()
    in_maps = make_in_maps(q, k, v, wq, wk, wv, wo, bq)
    res = run_bass_kernel_spmd(
        nc, in_maps, core_ids=list(range(NCORES)), trace=_trace
    )
    _CACHE["last_results"] = res
    outs = [res.results[c]["ot"] for c in range(NCORES)]
    return assemble(outs, bv, wo, bo)

